# revision 7
# baseline (speedup 1.0000x reference)
"""Trainium2 Bass kernel for nn_MessageAggregator (gnn_message_passing). v5

Computation (reference):
    s   = logsig(logsig(state @ W1_m.T + b1_m) @ W2_m.T)      # [E, D]
    agg = mask_transpose @ (mask @ s) - s                     # [E, D]
    out = logsig(logsig([agg, feature] @ W1_a.T + b1_a) @ W2_a.T)

Sharding: edge dimension E=32768 split across 8 cores (4096 edges each).
phase 0: memory-MLP (exact softplus via Exp+Ln, ACT stages gated so the
         tile scheduler cannot interleave tables: 6 loads total)
phase 1: v-partial via fp8e4 DoubleRow matmuls (256-edge pairs / pass)
AllReduce (bf16)
phase 2: bf16 vT (stationary) x fp8 mask (moving) matmuls; table-free MLP
         (softplus->relu, logsig->min(x,0)); fp16 output DMA.
All weights host-transposed/negated/cast.
"""

import ml_dtypes
import numpy as np

N_CORES = 8
E, N, D, DF = 32768, 2048, 128, 32
EL = E // N_CORES          # 4096 edges per core
NT = EL // 128             # 32 edge tiles of 128
NPAIR = NT // 2            # 16 DoubleRow edge pair-tiles
P = 128

_CACHE: dict = {}


def _build():
    from concourse import bacc, mybir, tile

    F32 = mybir.dt.float32
    BF16 = mybir.dt.bfloat16
    FP16 = mybir.dt.float16
    FP8 = mybir.dt.float8e4
    AF = mybir.ActivationFunctionType
    ALU = mybir.AluOpType
    DR = mybir.MatmulPerfMode.DoubleRow

    nc = bacc.Bacc("TRN2", target_bir_lowering=False, debug=False,
                   num_devices=N_CORES)

    stateT_l = nc.dram_tensor("stateT_l", [D, EL], BF16, kind="ExternalInput")
    featT_l = nc.dram_tensor("featT_l", [DF, EL], BF16, kind="ExternalInput")
    # mT pair-tiles: [pair, p, slot, node] with edge = pair*256 + slot*128 + p
    mTp_l = nc.dram_tensor("mTp_l", [NPAIR, P, 2, N], FP8, kind="ExternalInput")
    mask_l = nc.dram_tensor("mask_l", [N, EL], FP8, kind="ExternalInput")
    w1mT = nc.dram_tensor("w1mT", [D, D], BF16, kind="ExternalInput")
    w2mnT = nc.dram_tensor("w2mnT", [D, D], BF16, kind="ExternalInput")
    w1anT = nc.dram_tensor("w1anT", [D, D], BF16, kind="ExternalInput")
    wa2T = nc.dram_tensor("wa2T", [DF, D], BF16, kind="ExternalInput")
    w2anT = nc.dram_tensor("w2anT", [D, D], FP16, kind="ExternalInput")
    nb1m = nc.dram_tensor("nb1m", [D], F32, kind="ExternalInput")
    nb1a = nc.dram_tensor("nb1a", [D], F32, kind="ExternalInput")
    idn_b = nc.dram_tensor("idn_b", [P, P], BF16, kind="ExternalInput")
    out_l = nc.dram_tensor("out_l", [EL, D], FP16, kind="ExternalOutput")

    with tile.TileContext(nc) as tc:
        with (
            tc.tile_pool(name="consts", bufs=1) as consts,
            tc.tile_pool(name="persist", bufs=1) as persist,
            tc.tile_pool(name="mlp", bufs=4) as mlp,
            tc.tile_pool(name="mtp", bufs=16) as mtp,
            tc.tile_pool(name="streamp", bufs=18) as streamp,
            tc.tile_pool(name="outp", bufs=2) as outp,
            tc.tile_pool(name="ps_acc", bufs=1, space="PSUM") as ps_acc,
            tc.tile_pool(name="ps_mm", bufs=2, space="PSUM") as ps_mm,
            tc.tile_pool(name="ps_tp", bufs=2, space="PSUM") as ps_tp,
            tc.tile_pool(name="dram", bufs=1, space="DRAM") as dram,
        ):
            # ---------------- constants (host-prepped) ----------------
            w1mT_sb = consts.tile([D, D], BF16)
            nc.sync.dma_start(w1mT_sb[:], w1mT[:])
            w2mnT_sb = consts.tile([D, D], BF16)
            nc.sync.dma_start(w2mnT_sb[:], w2mnT[:])
            w1anT_sb = consts.tile([D, D], BF16)
            nc.sync.dma_start(w1anT_sb[:], w1anT[:])
            wa2T_sb = consts.tile([DF, D], BF16)
            nc.sync.dma_start(wa2T_sb[:], wa2T[:])
            w2anT_sb = consts.tile([D, D], FP16)
            nc.sync.dma_start(w2anT_sb[:], w2anT[:])
            nb1m_sb = consts.tile([D, 1], F32)
            nc.sync.dma_start(nb1m_sb[:], nb1m[:, None])
            nb1a_sb = consts.tile([D, 1], F32)
            nc.sync.dma_start(nb1a_sb[:], nb1a[:, None])
            idn_bf = consts.tile([P, P], BF16)
            nc.sync.dma_start(idn_bf[:], idn_b[:])

            # ---------------- persistent intermediates ----------------
            u2T = persist.tile([P, EL], BF16)      # -s.T (feat-major)
            u2e = persist.tile([P, NT, D], FP8)    # -s    (edge-major tiles)
            featT = persist.tile([DF, EL], BF16)   # feature.T
            vT = persist.tile([P, N // P, D], BF16)  # -agg  [n, da] tiles

            stateT_sb = persist.tile([P, EL], BF16)
            for q4 in range(4):
                nc.sync.dma_start(
                    stateT_sb[:, q4 * 1024 : (q4 + 1) * 1024],
                    stateT_l[:, q4 * 1024 : (q4 + 1) * 1024],
                )
            nc.sync.dma_start(featT[:], featT_l[:])

            # ------- phase 0 (memory MLP) interleaved with phase 1 -------
            # ACT stage gates (scheduler-time, ms): cluster same-table
            # stages; g1's Exp-L1 shares g0's Exp-L2 table load. Model time
            # excludes the runtime preamble, so natural stage times run
            # ~2-25us; gates must sit above them to bind.
            GATE = {
                ("E1", 0): 0.006, ("L1", 0): 0.012,
                ("E2", 0): 0.018, ("E1", 1): 0.018,
                ("L2", 0): 0.024, ("L1", 1): 0.024,
                ("E2", 1): 0.030, ("L2", 1): 0.036,
            }
            accs = [
                ps_acc.tile([P, 512], F32, tag=f"acc{q}", name=f"p1acc{q}")
                for q in range(4)
            ]

            h1s, ex1s, u1s, z2s, ex2s = {}, {}, {}, {}, {}

            def p0_mm1(cj):
                for j in cj:
                    h1 = ps_mm.tile([P, 512], F32, tag="mm", name=f"h1_{j}")
                    nc.tensor.matmul(
                        h1[:], w1mT_sb[:],
                        stateT_sb[:, j * 512 : (j + 1) * 512],
                        start=True, stop=True,
                    )
                    h1s[j] = h1

            def p0_exp1(cj, grp):
                with tc.tile_wait_until(GATE[("E1", grp)]):
                    for j in cj:
                        ex1 = mlp.tile([P, 512], F32, tag="ex1",
                                       name=f"ex1_{j}")
                        nc.scalar.activation(ex1[:], h1s[j][:], AF.Exp,
                                             scale=-1.0, bias=nb1m_sb[:])
                        ex1s[j] = ex1

            def p0_ln1(cj, grp):
                with tc.tile_wait_until(GATE[("L1", grp)]):
                    for j in cj:
                        u1 = mlp.tile([P, 512], BF16, tag="u1",
                                      name=f"u1_{j}")
                        nc.scalar.activation(u1[:], ex1s[j][:], AF.Ln,
                                             bias=1.0)
                        u1s[j] = u1

            def p0_mm2(cj):
                for j in cj:
                    z2 = ps_mm.tile([P, 512], F32, tag="mm", name=f"z2_{j}")
                    nc.tensor.matmul(z2[:], w2mnT_sb[:], u1s[j][:],
                                     start=True, stop=True)
                    z2s[j] = z2

            def p0_exp2(cj, grp):
                with tc.tile_wait_until(GATE[("E2", grp)]):
                    for j in cj:
                        ex2 = mlp.tile([P, 512], F32, tag="ex2",
                                       name=f"ex2_{j}")
                        nc.scalar.activation(ex2[:], z2s[j][:], AF.Exp,
                                             scale=-1.0)
                        ex2s[j] = ex2

            def p0_ln2(cj, grp):
                with tc.tile_wait_until(GATE[("L2", grp)]):
                    for j in cj:
                        nc.scalar.activation(
                            u2T[:, j * 512 : (j + 1) * 512], ex2s[j][:],
                            AF.Ln, bias=1.0,
                        )

            mts = {}

            def p0_tail_p1(cj):
                # transposes to edge-major fp8 + DoubleRow phase-1 matmuls
                # (node half 0 only; mt pair-tiles stay resident for half 1)
                for j in cj:
                    tp2 = ps_tp.tile([P, 512], BF16, tag="tp",
                                     name=f"tp2_{j}")
                    for k in range(4):
                        c0 = (j * 4 + k) * P
                        nc.tensor.transpose(
                            tp2[:, k * P : (k + 1) * P],
                            u2T[:, c0 : c0 + P],
                            idn_bf[:],
                        )
                    nc.vector.tensor_copy(
                        u2e[:, j * 4 : (j + 1) * 4, :].rearrange(
                            "p a d -> p (a d)"
                        ),
                        tp2[:],
                    )
                    for pr in range(2 * j, 2 * j + 2):
                        mt = mtp.tile([P, 2, N], FP8, tag="mt",
                                      name=f"mt_{pr}")
                        nc.sync.dma_start(mt[:, :, :], mTp_l[pr, :, :, :])
                        mts[pr] = mt
                        for q in range(2):
                            nc.tensor.matmul(
                                accs[q][:],
                                u2e[:, 2 * pr : 2 * pr + 2, :],
                                mt[:, :, q * 512 : (q + 1) * 512],
                                start=(pr == 0),
                                stop=(pr == NPAIR - 1),
                                perf_mode=DR,
                            )

            for grp in range(2):
                cj = [4 * grp + i for i in range(4)]
                p0_mm1(cj)
                p0_exp1(cj, grp)
                p0_ln1(cj, grp)
                p0_mm2(cj)
                p0_exp2(cj, grp)
                p0_ln2(cj, grp)
                p0_tail_p1(cj)

            # ---------------- chunked AllReduce (2 node halves) ----------
            vsb = persist.tile([P, N], BF16)
            cc_ins = [dram.tile([P, N // 2], BF16, name=f"cc_in{h}")
                      for h in range(2)]
            cc_outs = [dram.tile([P, N // 2], BF16, addr_space="Shared",
                                 name=f"cc_out{h}")
                       for h in range(2)]
            vfull = persist.tile([P, N], BF16)

            def fire_ar(h):
                for q in (2 * h, 2 * h + 1):
                    nc.vector.tensor_copy(
                        vsb[:, q * 512 : (q + 1) * 512], accs[q][:]
                    )
                nc.gpsimd.dma_start(
                    cc_ins[h][:], vsb[:, h * 1024 : (h + 1) * 1024]
                )
                nc.gpsimd.collective_compute(
                    "AllReduce",
                    mybir.AluOpType.add,
                    ins=[cc_ins[h].opt()],
                    outs=[cc_outs[h].opt()],
                    replica_groups=[list(range(N_CORES))],
                )
                for hv in range(2):
                    nc.gpsimd.dma_start(
                        vfull[:, h * 1024 + hv * 512 : h * 1024 + (hv + 1) * 512],
                        cc_outs[h][:, hv * 512 : (hv + 1) * 512],
                    )
                for g in range(2):
                    tp3 = ps_tp.tile([P, 512], BF16, tag="tp",
                                     name=f"tp3_{h}_{g}")
                    for k in range(4):
                        i = (2 * h + g) * 4 + k
                        nc.tensor.transpose(
                            tp3[:, k * P : (k + 1) * P],
                            vfull[:, i * P : (i + 1) * P],
                            idn_bf[:],
                        )
                    nc.vector.tensor_copy(
                        vT[:, (2 * h + g) * 4 : (2 * h + g + 1) * 4, :]
                        .rearrange("p a d -> p (a d)"),
                        tp3[:],
                    )

            # half 0 accumulation finished above -> AR0; then half 1 mms
            # (mt tiles resident) -> AR1
            fire_ar(0)
            for pr in range(NPAIR):
                for q in (2, 3):
                    nc.tensor.matmul(
                        accs[q][:],
                        u2e[:, 2 * pr : 2 * pr + 2, :],
                        mts[pr][:, :, q * 512 : (q + 1) * 512],
                        start=(pr == 0),
                        stop=(pr == NPAIR - 1),
                        perf_mode=DR,
                    )
            fire_ar(1)

            # ---------------- phase 2: edge agg + concat MLP ----------------
            # table-free MLP: u3 = relu(-z1a - b1a), out = min(po, 0)
            out_v = out_l.rearrange("(c k p) d -> c p k d", k=4, p=P)

            def p2_mlp(jacc):
                w3s, z1as, u3s, pos = {}, {}, {}, {}
                for j, acc in jacc:
                    w3 = mlp.tile([P, 512], BF16, tag="w3", name=f"w3_{j}")
                    nc.vector.tensor_sub(
                        w3[:], acc[:], u2T[:, j * 512 : (j + 1) * 512]
                    )
                    w3s[j] = w3
                for j, acc in jacc:
                    z1a = ps_mm.tile([P, 512], F32, tag="mm", name=f"z1a_{j}")
                    nc.tensor.matmul(z1a[:], w1anT_sb[:], w3s[j][:],
                                     start=True, stop=False)
                    nc.tensor.matmul(
                        z1a[:], wa2T_sb[:], featT[:, j * 512 : (j + 1) * 512],
                        start=False, stop=True,
                    )
                    z1as[j] = z1a
                for j, acc in jacc:
                    u3 = mlp.tile([P, 512], FP16, tag="u3", name=f"u3_{j}")
                    nc.scalar.activation(u3[:], z1as[j][:], AF.Relu,
                                         scale=-1.0, bias=nb1a_sb[:])
                    u3s[j] = u3
                for j, acc in jacc:
                    po = ps_tp.tile([P, 512], F32, tag="tp", name=f"po_{j}")
                    for k in range(4):
                        nc.tensor.matmul(
                            po[:, k * P : (k + 1) * P],
                            u3s[j][:, k * P : (k + 1) * P],
                            w2anT_sb[:],
                            start=True,
                            stop=True,
                        )
                    pos[j] = po
                for j, acc in jacc:
                    ob = outp.tile([P, 512], FP16, tag="ob", name=f"ob_{j}")
                    nc.vector.tensor_scalar(
                        ob[:], pos[j][:], 0.0, None, ALU.min
                    )
                    nc.gpsimd.dma_start(
                        out_v[j], ob.rearrange("p (k d) -> p k d", k=4)
                    )

            # 2 waves of 2048 edges; [128,2048] fp8 mask tiles (2KB lines)
            for w in range(2):
                js = [4 * w + i for i in range(4)]
                acc_w = {
                    j: ps_acc.tile([P, 512], F32, tag=f"acc{j % 4}",
                                   name=f"p2acc_{j}")
                    for j in js
                }
                for nch in range(16):
                    mk = streamp.tile([P, 2048], FP8, tag="sp",
                                      name=f"mk_{w}_{nch}")
                    nc.sync.dma_start(
                        mk[:],
                        mask_l[
                            nch * P : (nch + 1) * P,
                            w * 2048 : (w + 1) * 2048,
                        ],
                    )
                    for ji, j in enumerate(js):
                        nc.tensor.matmul(
                            acc_w[j][:],
                            vT[:, nch, :],
                            mk[:, ji * 512 : (ji + 1) * 512],
                            start=(nch == 0),
                            stop=(nch == 15),
                        )
                p2_mlp([(j, acc_w[j]) for j in js])
    nc.compile()
    return nc


def kernel(**inputs: np.ndarray) -> np.ndarray:
    from concourse.bass_utils import run_bass_kernel_spmd

    if "nc" not in _CACHE:
        _CACHE["nc"] = _build()
    nc = _CACHE["nc"]

    state = np.ascontiguousarray(inputs["state"], dtype=np.float32)
    feature = np.ascontiguousarray(inputs["feature"], dtype=np.float32)
    mask = np.ascontiguousarray(inputs["mask"], dtype=np.float32)
    mask_transpose = np.ascontiguousarray(
        inputs["mask_transpose"], dtype=np.float32
    )

    W1m = np.asarray(inputs["W1_m"], dtype=np.float32)
    W2m = np.asarray(inputs["W2_m"], dtype=np.float32)
    W1a = np.asarray(inputs["W1_a"], dtype=np.float32)
    W2a = np.asarray(inputs["W2_a"], dtype=np.float32)
    common = {
        "w1mT": np.ascontiguousarray(W1m.T).astype(ml_dtypes.bfloat16),
        "w2mnT": np.ascontiguousarray(-W2m.T).astype(ml_dtypes.bfloat16),
        "w1anT": np.ascontiguousarray(-W1a[:, :D].T).astype(
            ml_dtypes.bfloat16
        ),
        "wa2T": np.ascontiguousarray(W1a[:, D:].T).astype(ml_dtypes.bfloat16),
        "w2anT": np.ascontiguousarray(-W2a.T).astype(np.float16),
        "nb1m": -np.asarray(inputs["b1_m"], dtype=np.float32),
        "nb1a": -np.asarray(inputs["b1_a"], dtype=np.float32),
        "idn_b": np.eye(P, dtype=np.float32).astype(ml_dtypes.bfloat16),
    }
    in_maps = []
    for c in range(N_CORES):
        sl = slice(c * EL, (c + 1) * EL)
        mtp_h = (
            mask_transpose[sl]
            .reshape(NPAIR, 2, P, N)
            .transpose(0, 2, 1, 3)
        )
        in_maps.append(
            {
                "stateT_l": np.ascontiguousarray(state[sl].T).astype(
                    ml_dtypes.bfloat16
                ),
                "featT_l": np.ascontiguousarray(feature[sl].T).astype(
                    ml_dtypes.bfloat16
                ),
                "mTp_l": np.ascontiguousarray(mtp_h).astype(
                    ml_dtypes.float8_e4m3fn
                ),
                "mask_l": np.ascontiguousarray(mask[:, sl]).astype(
                    ml_dtypes.float8_e4m3fn
                ),
                **common,
            }
        )
    _CACHE["in_maps"] = in_maps

    res = run_bass_kernel_spmd(nc, in_maps, core_ids=list(range(N_CORES)))
    out = np.concatenate(
        [np.asarray(res.results[c]["out_l"]).astype(np.float32)
         for c in range(N_CORES)],
        axis=0,
    )
    return out


# revision 8
# speedup vs baseline: 1.0498x; 1.0498x over previous
"""Trainium2 Bass kernel for nn_MessageAggregator (gnn_message_passing). v5

Computation (reference):
    s   = logsig(logsig(state @ W1_m.T + b1_m) @ W2_m.T)      # [E, D]
    agg = mask_transpose @ (mask @ s) - s                     # [E, D]
    out = logsig(logsig([agg, feature] @ W1_a.T + b1_a) @ W2_a.T)

Sharding: edge dimension E=32768 split across 8 cores (4096 edges each).
phase 0: memory-MLP (exact softplus via Exp+Ln, ACT stages gated so the
         tile scheduler cannot interleave tables: 6 loads total)
phase 1: v-partial via fp8e4 DoubleRow matmuls (256-edge pairs / pass)
AllReduce (bf16)
phase 2: bf16 vT (stationary) x fp8 mask (moving) matmuls; table-free MLP
         (softplus->relu, logsig->min(x,0)); fp16 output DMA.
All weights host-transposed/negated/cast.
"""

import ml_dtypes
import numpy as np

N_CORES = 8
E, N, D, DF = 32768, 2048, 128, 32
EL = E // N_CORES          # 4096 edges per core
NT = EL // 128             # 32 edge tiles of 128
NPAIR = NT // 2            # 16 DoubleRow edge pair-tiles
P = 128

_CACHE: dict = {}


def _build():
    from concourse import bacc, mybir, tile

    F32 = mybir.dt.float32
    BF16 = mybir.dt.bfloat16
    FP16 = mybir.dt.float16
    FP8 = mybir.dt.float8e4
    AF = mybir.ActivationFunctionType
    ALU = mybir.AluOpType
    DR = mybir.MatmulPerfMode.DoubleRow

    nc = bacc.Bacc("TRN2", target_bir_lowering=False, debug=False,
                   num_devices=N_CORES)

    stateT_l = nc.dram_tensor("stateT_l", [D, EL], BF16, kind="ExternalInput")
    featT_l = nc.dram_tensor("featT_l", [DF, EL], BF16, kind="ExternalInput")
    # mT pair-tiles: [pair, p, slot, node] with edge = pair*256 + slot*128 + p
    mTp_l = nc.dram_tensor("mTp_l", [NPAIR, P, 2, N], FP8, kind="ExternalInput")
    mask_l = nc.dram_tensor("mask_l", [N, EL], FP8, kind="ExternalInput")
    w1mT = nc.dram_tensor("w1mT", [D, D], BF16, kind="ExternalInput")
    w2mnT = nc.dram_tensor("w2mnT", [D, D], BF16, kind="ExternalInput")
    w1anT = nc.dram_tensor("w1anT", [D, D], BF16, kind="ExternalInput")
    wa2T = nc.dram_tensor("wa2T", [DF, D], BF16, kind="ExternalInput")
    w2anT = nc.dram_tensor("w2anT", [D, D], FP16, kind="ExternalInput")
    nb1m = nc.dram_tensor("nb1m", [D], F32, kind="ExternalInput")
    nb1a = nc.dram_tensor("nb1a", [D], F32, kind="ExternalInput")
    idn_b = nc.dram_tensor("idn_b", [P, P], BF16, kind="ExternalInput")
    out_l = nc.dram_tensor("out_l", [EL, D], FP16, kind="ExternalOutput")

    with tile.TileContext(nc) as tc:
        with (
            tc.tile_pool(name="consts", bufs=1) as consts,
            tc.tile_pool(name="persist", bufs=1) as persist,
            tc.tile_pool(name="mlp", bufs=4) as mlp,
            tc.tile_pool(name="mtp", bufs=16) as mtp,
            tc.tile_pool(name="streamp", bufs=24) as streamp,
            tc.tile_pool(name="outp", bufs=2) as outp,
            tc.tile_pool(name="ps_acc", bufs=1, space="PSUM") as ps_acc,
            tc.tile_pool(name="ps_mm", bufs=2, space="PSUM") as ps_mm,
            tc.tile_pool(name="ps_tp", bufs=2, space="PSUM") as ps_tp,
            tc.tile_pool(name="dram", bufs=1, space="DRAM") as dram,
        ):
            # ---------------- constants (host-prepped) ----------------
            w1mT_sb = consts.tile([D, D], BF16)
            nc.sync.dma_start(w1mT_sb[:], w1mT[:])
            w2mnT_sb = consts.tile([D, D], BF16)
            nc.sync.dma_start(w2mnT_sb[:], w2mnT[:])
            w1anT_sb = consts.tile([D, D], BF16)
            nc.sync.dma_start(w1anT_sb[:], w1anT[:])
            wa2T_sb = consts.tile([DF, D], BF16)
            nc.sync.dma_start(wa2T_sb[:], wa2T[:])
            w2anT_sb = consts.tile([D, D], FP16)
            nc.sync.dma_start(w2anT_sb[:], w2anT[:])
            nb1m_sb = consts.tile([D, 1], F32)
            nc.sync.dma_start(nb1m_sb[:], nb1m[:, None])
            nb1a_sb = consts.tile([D, 1], F32)
            nc.sync.dma_start(nb1a_sb[:], nb1a[:, None])
            idn_bf = consts.tile([P, P], BF16)
            nc.sync.dma_start(idn_bf[:], idn_b[:])

            # ---------------- persistent intermediates ----------------
            u2T = persist.tile([P, EL], BF16)      # -s.T (feat-major)
            u2e = persist.tile([P, NT, D], FP8)    # -s    (edge-major tiles)
            featT = persist.tile([DF, EL], BF16)   # feature.T
            vT = persist.tile([P, N // P, D], BF16)  # -agg  [n, da] tiles

            stateT_sb = persist.tile([P, EL], BF16)
            for q4 in range(4):
                nc.sync.dma_start(
                    stateT_sb[:, q4 * 1024 : (q4 + 1) * 1024],
                    stateT_l[:, q4 * 1024 : (q4 + 1) * 1024],
                )
            nc.sync.dma_start(featT[:], featT_l[:])

            # ------- phase 0 (memory MLP) interleaved with phase 1 -------
            # ACT stage gates (scheduler-time, ms): cluster same-table
            # stages; g1's Exp-L1 shares g0's Exp-L2 table load. Model time
            # excludes the runtime preamble, so natural stage times run
            # ~2-25us; gates must sit above them to bind.
            GATE = {
                ("E1", 0): 0.006, ("L1", 0): 0.012,
                ("E2", 0): 0.018, ("E1", 1): 0.018,
                ("L2", 0): 0.024, ("L1", 1): 0.024,
                ("E2", 1): 0.030, ("L2", 1): 0.036,
            }
            accs = [
                ps_acc.tile([P, 512], F32, tag=f"acc{q}", name=f"p1acc{q}")
                for q in range(4)
            ]

            h1s, ex1s, u1s, z2s, ex2s = {}, {}, {}, {}, {}

            def p0_mm1(cj):
                for j in cj:
                    h1 = ps_mm.tile([P, 512], F32, tag="mm", name=f"h1_{j}")
                    nc.tensor.matmul(
                        h1[:], w1mT_sb[:],
                        stateT_sb[:, j * 512 : (j + 1) * 512],
                        start=True, stop=True,
                    )
                    h1s[j] = h1

            def p0_exp1(cj, grp):
                with tc.tile_wait_until(GATE[("E1", grp)]):
                    for j in cj:
                        ex1 = mlp.tile([P, 512], F32, tag="ex1",
                                       name=f"ex1_{j}")
                        nc.scalar.activation(ex1[:], h1s[j][:], AF.Exp,
                                             scale=-1.0, bias=nb1m_sb[:])
                        ex1s[j] = ex1

            def p0_ln1(cj, grp):
                with tc.tile_wait_until(GATE[("L1", grp)]):
                    for j in cj:
                        u1 = mlp.tile([P, 512], BF16, tag="u1",
                                      name=f"u1_{j}")
                        nc.scalar.activation(u1[:], ex1s[j][:], AF.Ln,
                                             bias=1.0)
                        u1s[j] = u1

            def p0_mm2(cj):
                for j in cj:
                    z2 = ps_mm.tile([P, 512], F32, tag="mm", name=f"z2_{j}")
                    nc.tensor.matmul(z2[:], w2mnT_sb[:], u1s[j][:],
                                     start=True, stop=True)
                    z2s[j] = z2

            def p0_exp2(cj, grp):
                with tc.tile_wait_until(GATE[("E2", grp)]):
                    for j in cj:
                        ex2 = mlp.tile([P, 512], F32, tag="ex2",
                                       name=f"ex2_{j}")
                        nc.scalar.activation(ex2[:], z2s[j][:], AF.Exp,
                                             scale=-1.0)
                        ex2s[j] = ex2

            def p0_ln2(cj, grp):
                with tc.tile_wait_until(GATE[("L2", grp)]):
                    for j in cj:
                        nc.scalar.activation(
                            u2T[:, j * 512 : (j + 1) * 512], ex2s[j][:],
                            AF.Ln, bias=1.0,
                        )

            mts = {}

            def p0_tail_p1(cj):
                # transposes to edge-major fp8 + DoubleRow phase-1 matmuls
                # (node half 0 only; mt pair-tiles stay resident for half 1)
                for j in cj:
                    tp2 = ps_tp.tile([P, 512], BF16, tag="tp",
                                     name=f"tp2_{j}")
                    for k in range(4):
                        c0 = (j * 4 + k) * P
                        nc.tensor.transpose(
                            tp2[:, k * P : (k + 1) * P],
                            u2T[:, c0 : c0 + P],
                            idn_bf[:],
                        )
                    nc.vector.tensor_copy(
                        u2e[:, j * 4 : (j + 1) * 4, :].rearrange(
                            "p a d -> p (a d)"
                        ),
                        tp2[:],
                    )
                    for pr in range(2 * j, 2 * j + 2):
                        mt = mtp.tile([P, 2, N], FP8, tag="mt",
                                      name=f"mt_{pr}")
                        nc.sync.dma_start(mt[:, :, :], mTp_l[pr, :, :, :])
                        mts[pr] = mt
                        for q in range(2):
                            nc.tensor.matmul(
                                accs[q][:],
                                u2e[:, 2 * pr : 2 * pr + 2, :],
                                mt[:, :, q * 512 : (q + 1) * 512],
                                start=(pr == 0),
                                stop=(pr == NPAIR - 1),
                                perf_mode=DR,
                            )

            for grp in range(2):
                cj = [4 * grp + i for i in range(4)]
                p0_mm1(cj)
                p0_exp1(cj, grp)
                p0_ln1(cj, grp)
                p0_mm2(cj)
                p0_exp2(cj, grp)
                p0_ln2(cj, grp)
                p0_tail_p1(cj)

            # ---------------- chunked AllReduce (2 node halves) ----------
            vsb = persist.tile([P, N], BF16)
            cc_ins = [dram.tile([P, N // 2], BF16, name=f"cc_in{h}")
                      for h in range(2)]
            cc_outs = [dram.tile([P, N // 2], BF16, addr_space="Shared",
                                 name=f"cc_out{h}")
                       for h in range(2)]
            vfull = persist.tile([P, N], BF16)

            def fire_ar(h):
                for q in (2 * h, 2 * h + 1):
                    nc.vector.tensor_copy(
                        vsb[:, q * 512 : (q + 1) * 512], accs[q][:]
                    )
                nc.gpsimd.dma_start(
                    cc_ins[h][:], vsb[:, h * 1024 : (h + 1) * 1024]
                )
                nc.gpsimd.collective_compute(
                    "AllReduce",
                    mybir.AluOpType.add,
                    ins=[cc_ins[h].opt()],
                    outs=[cc_outs[h].opt()],
                    replica_groups=[list(range(N_CORES))],
                )
                for hv in range(2):
                    nc.gpsimd.dma_start(
                        vfull[:, h * 1024 + hv * 512 : h * 1024 + (hv + 1) * 512],
                        cc_outs[h][:, hv * 512 : (hv + 1) * 512],
                    )
                for g in range(2):
                    tp3 = ps_tp.tile([P, 512], BF16, tag="tp",
                                     name=f"tp3_{h}_{g}")
                    for k in range(4):
                        i = (2 * h + g) * 4 + k
                        nc.tensor.transpose(
                            tp3[:, k * P : (k + 1) * P],
                            vfull[:, i * P : (i + 1) * P],
                            idn_bf[:],
                        )
                    nc.vector.tensor_copy(
                        vT[:, (2 * h + g) * 4 : (2 * h + g + 1) * 4, :]
                        .rearrange("p a d -> p (a d)"),
                        tp3[:],
                    )

            # half 0 accumulation finished above -> AR0; then half 1 mms
            # (mt tiles resident) -> AR1
            fire_ar(0)
            for pr in range(NPAIR):
                for q in (2, 3):
                    nc.tensor.matmul(
                        accs[q][:],
                        u2e[:, 2 * pr : 2 * pr + 2, :],
                        mts[pr][:, :, q * 512 : (q + 1) * 512],
                        start=(pr == 0),
                        stop=(pr == NPAIR - 1),
                        perf_mode=DR,
                    )
            fire_ar(1)

            # ---------------- phase 2: edge agg + concat MLP ----------------
            # table-free MLP: u3 = relu(-z1a - b1a), out = min(po, 0)
            out_v = out_l.rearrange("(c k p) d -> c p k d", k=4, p=P)

            def p2_mlp(jacc):
                w3s, z1as, u3s, pos = {}, {}, {}, {}
                for j, acc in jacc:
                    w3 = mlp.tile([P, 512], BF16, tag="w3", name=f"w3_{j}")
                    nc.vector.tensor_sub(
                        w3[:], acc[:], u2T[:, j * 512 : (j + 1) * 512]
                    )
                    w3s[j] = w3
                for j, acc in jacc:
                    z1a = ps_mm.tile([P, 512], F32, tag="mm", name=f"z1a_{j}")
                    nc.tensor.matmul(z1a[:], w1anT_sb[:], w3s[j][:],
                                     start=True, stop=False)
                    nc.tensor.matmul(
                        z1a[:], wa2T_sb[:], featT[:, j * 512 : (j + 1) * 512],
                        start=False, stop=True,
                    )
                    z1as[j] = z1a
                for j, acc in jacc:
                    u3 = mlp.tile([P, 512], FP16, tag="u3", name=f"u3_{j}")
                    nc.scalar.activation(u3[:], z1as[j][:], AF.Relu,
                                         scale=-1.0, bias=nb1a_sb[:])
                    u3s[j] = u3
                for j, acc in jacc:
                    po = ps_tp.tile([P, 512], F32, tag="tp", name=f"po_{j}")
                    for k in range(4):
                        nc.tensor.matmul(
                            po[:, k * P : (k + 1) * P],
                            u3s[j][:, k * P : (k + 1) * P],
                            w2anT_sb[:],
                            start=True,
                            stop=True,
                        )
                    pos[j] = po
                for j, acc in jacc:
                    ob = outp.tile([P, 512], FP16, tag="ob", name=f"ob_{j}")
                    nc.vector.tensor_scalar(
                        ob[:], pos[j][:], 0.0, None, ALU.min
                    )
                    nc.gpsimd.dma_start(
                        out_v[j], ob.rearrange("p (k d) -> p k d", k=4)
                    )

            # 2 waves of 2048 edges; [128,2048] fp8 mask tiles (2KB lines)
            for w in range(2):
                js = [4 * w + i for i in range(4)]
                acc_w = {
                    j: ps_acc.tile([P, 512], F32, tag=f"acc{j % 4}",
                                   name=f"p2acc_{j}")
                    for j in js
                }
                for nch in range(16):
                    mk = streamp.tile([P, 2048], FP8, tag="sp",
                                      name=f"mk_{w}_{nch}")
                    # gate the mask prefetch DMAs behind the phase-1 mT
                    # stream in the queues: these bytes are only needed
                    # after the mesh, and stealing bandwidth during
                    # phase 1 delays the collective arrival 1:1
                    with tc.tile_wait_until(0.06):
                        nc.sync.dma_start(
                            mk[:],
                            mask_l[
                                nch * P : (nch + 1) * P,
                                w * 2048 : (w + 1) * 2048,
                            ],
                        )
                    for ji, j in enumerate(js):
                        nc.tensor.matmul(
                            acc_w[j][:],
                            vT[:, nch, :],
                            mk[:, ji * 512 : (ji + 1) * 512],
                            start=(nch == 0),
                            stop=(nch == 15),
                        )
                p2_mlp([(j, acc_w[j]) for j in js])
    nc.compile()
    return nc


def kernel(**inputs: np.ndarray) -> np.ndarray:
    from concourse.bass_utils import run_bass_kernel_spmd

    if "nc" not in _CACHE:
        _CACHE["nc"] = _build()
    nc = _CACHE["nc"]

    state = np.ascontiguousarray(inputs["state"], dtype=np.float32)
    feature = np.ascontiguousarray(inputs["feature"], dtype=np.float32)
    mask = np.ascontiguousarray(inputs["mask"], dtype=np.float32)
    mask_transpose = np.ascontiguousarray(
        inputs["mask_transpose"], dtype=np.float32
    )

    W1m = np.asarray(inputs["W1_m"], dtype=np.float32)
    W2m = np.asarray(inputs["W2_m"], dtype=np.float32)
    W1a = np.asarray(inputs["W1_a"], dtype=np.float32)
    W2a = np.asarray(inputs["W2_a"], dtype=np.float32)
    common = {
        "w1mT": np.ascontiguousarray(W1m.T).astype(ml_dtypes.bfloat16),
        "w2mnT": np.ascontiguousarray(-W2m.T).astype(ml_dtypes.bfloat16),
        "w1anT": np.ascontiguousarray(-W1a[:, :D].T).astype(
            ml_dtypes.bfloat16
        ),
        "wa2T": np.ascontiguousarray(W1a[:, D:].T).astype(ml_dtypes.bfloat16),
        "w2anT": np.ascontiguousarray(-W2a.T).astype(np.float16),
        "nb1m": -np.asarray(inputs["b1_m"], dtype=np.float32),
        "nb1a": -np.asarray(inputs["b1_a"], dtype=np.float32),
        "idn_b": np.eye(P, dtype=np.float32).astype(ml_dtypes.bfloat16),
    }
    in_maps = []
    for c in range(N_CORES):
        sl = slice(c * EL, (c + 1) * EL)
        mtp_h = (
            mask_transpose[sl]
            .reshape(NPAIR, 2, P, N)
            .transpose(0, 2, 1, 3)
        )
        in_maps.append(
            {
                "stateT_l": np.ascontiguousarray(state[sl].T).astype(
                    ml_dtypes.bfloat16
                ),
                "featT_l": np.ascontiguousarray(feature[sl].T).astype(
                    ml_dtypes.bfloat16
                ),
                "mTp_l": np.ascontiguousarray(mtp_h).astype(
                    ml_dtypes.float8_e4m3fn
                ),
                "mask_l": np.ascontiguousarray(mask[:, sl]).astype(
                    ml_dtypes.float8_e4m3fn
                ),
                **common,
            }
        )
    _CACHE["in_maps"] = in_maps

    res = run_bass_kernel_spmd(nc, in_maps, core_ids=list(range(N_CORES)))
    out = np.concatenate(
        [np.asarray(res.results[c]["out_l"]).astype(np.float32)
         for c in range(N_CORES)],
        axis=0,
    )
    return out
